# revision 16
# baseline (speedup 1.0000x reference)
"""Block-sparse MoE (top-2 of 8 experts, SwiGLU) for Trainium2, 8 NeuronCores.

Strategy: data-parallel over tokens (2048 tokens/core, no collectives),
with on-device routing and capacity-based sparse dispatch per core:

  1. Router: logits = x @ gate_w.T in fp32 on PE; top-2 via DVE max8;
     renormalized top-2 softmax weights computed as sigmoid(l_i - l_j).
  2. Dispatch: per-expert running ranks via a masked prefix-scan over the
     [8, 4096] one-hot pair matrix; slot id d = expert*CAP + rank; the
     inverse permutation (slot -> token) is built with an indirect-DMA
     scatter of token ids; tokens are gathered by row with indirect DMA
     and transposed on PE into [H, slot] layout for the FFN matmuls.
  3. FFN per expert (CAP=640 slots): hT = silu(w1 @ xgT) * (w3 @ xgT),
     yg = (hT.T @ w2.T) -> [slot, H] rows written to DRAM scratch.
  4. Combine: gather yg rows at each token's two slots, out = wA*yA + wB*yB.

Matmul compute dtype is a knob (bf16 / fp32 / f32r); router is always fp32.
"""
import os
import sys

if "/opt/trn_rl_repo" not in sys.path:
    sys.path.insert(0, "/opt/trn_rl_repo")

import numpy as np
import ml_dtypes

import concourse.bacc as bacc
import concourse.bass as bass
import concourse.mybir as mybir
import concourse.tile as tile
from concourse.bass import ts
from concourse.bass_utils import run_bass_kernel_spmd
from concourse.masks import make_identity

dt = mybir.dt

# ---- problem constants (hardcoded per spec) ----
B, S, H, F, E = 4, 4096, 1024, 2048, 8
T = B * S                  # 16384 tokens
NCORES = 8
TC = T // NCORES           # 2048 tokens per core
NT = TC // 128             # 16 token tiles
NPAIR = 2 * TC // 128      # 32 pair tiles
CAP = 640                  # per-(core,expert) slot capacity (max count is 565)
STE = CAP // 128           # 5 slot tiles per expert
SLOTS = E * CAP            # 5120
NCH = 2                    # slot chunks for stage-A psum (N<=512)
CHUNK = CAP // NCH         # 320
KH = H // 128              # 8 k-tiles over H
KF = F // 128              # 16 k-tiles over F
FT = F // 128              # 16 f tiles

MM_MODE = os.environ.get("MOE_MM_MODE", "bf16")  # bf16 | f32 | f32r
USE_SILU = os.environ.get("MOE_USE_SILU", "1") == "1"  # 0: sigmoid*x (sim-safe)
ACC_DT = dt.float32

if MM_MODE == "bf16":
    MM_DT = dt.bfloat16
    MM_NP = ml_dtypes.bfloat16
    NFH = 4                # F-slices for stage-A weight streaming
    NW2 = 2                # H-slices for stage-B weight streaming
    BIG_BUFS = 2           # xgt/ht double buffering
else:
    MM_DT = dt.float32
    MM_NP = np.float32
    NFH = 8
    NW2 = 4
    BIG_BUFS = 1
FSL = F // NFH             # stage-A weight slice width (f)
HSL = H // NW2             # stage-B weight slice width (h)


def _mm_cast(ap):
    """Bitcast fp32 APs to float32r for fast fp32 matmul when requested."""
    if MM_MODE == "f32r":
        return ap.bitcast(dt.float32r)
    return ap


def build_nc():
    nc = bacc.Bacc("TRN2", target_bir_lowering=False, debug=False)

    # ---- I/O ----
    xt_d = nc.dram_tensor("xt", [128, KH, TC], dt.float32, kind="ExternalInput").ap()
    xb_d = nc.dram_tensor("xb", [TC, H], MM_DT, kind="ExternalInput").ap()
    gwt_d = nc.dram_tensor("gwt", [128, KH, E], dt.float32, kind="ExternalInput").ap()
    w1_d = nc.dram_tensor("w1s", [E, NFH, 128, KH, FSL], MM_DT, kind="ExternalInput").ap()
    w3_d = nc.dram_tensor("w3s", [E, NFH, 128, KH, FSL], MM_DT, kind="ExternalInput").ap()
    w2_d = nc.dram_tensor("w2s", [E, NW2, 128, KF, HSL], MM_DT, kind="ExternalInput").ap()
    out_d = nc.dram_tensor("out", [TC, H], dt.float32, kind="ExternalOutput").ap()

    # ---- DRAM scratch ----
    eall_d = nc.dram_tensor("eall", [2 * TC], dt.uint32).ap()
    dall_d = nc.dram_tensor("dall", [2 * TC], dt.int32).ap()
    src_d = nc.dram_tensor("srcd", [SLOTS, 1], dt.int32).ap()
    yg_d = nc.dram_tensor("ygd", [SLOTS, H], dt.float32).ap()

    with tile.TileContext(nc) as tc:
        _emit(tc, nc, xt_d, xb_d, gwt_d, w1_d, w3_d, w2_d, out_d,
              eall_d, dall_d, src_d, yg_d)
    nc.compile()
    return nc


def _emit(tc, nc, xt_d, xb_d, gwt_d, w1_d, w3_d, w2_d, out_d,
          eall_d, dall_d, src_d, yg_d):
    AF = mybir.ActivationFunctionType
    OP = mybir.AluOpType

    _pools = []

    def _pool(**kw):
        p = tc.alloc_tile_pool(**kw)
        _pools.append(p)
        return p

    res = _pool(name="resident", bufs=1)
    # resident small tiles
    wcomb = res.tile([128, NT, 2], dt.float32)      # per-token top-2 weights
    dp_sb = res.tile([128, NPAIR], dt.int32)        # pair-major slot ids
    src_sb = res.tile([128, E * STE], dt.int32)     # slot-major source tokens
    ident = res.tile([128, 128], MM_DT)
    make_identity(nc, ident[:])

    # =================== phase 1: router ===================
    with tc.tile_pool(name="router", bufs=1) as rp, \
         tc.tile_pool(name="rsmall", bufs=4) as rs, \
         tc.tile_pool(name="rpsum", bufs=2, space="PSUM") as rps:
        xt = rp.tile([128, KH, TC], dt.float32)
        nc.sync.dma_start(xt[:], xt_d[:])
        gwt = rp.tile([128, KH, E], dt.float32)
        nc.sync.dma_start(gwt[:], gwt_d[:])

        for tt in range(NT):
            psl = rps.tile([128, E], dt.float32, space="PSUM")
            for k in range(KH):
                nc.tensor.matmul(psl[:], lhsT=xt[:, k, ts(tt, 128)],
                                 rhs=gwt[:, k, :], start=(k == 0), stop=(k == KH - 1))
            lg = rs.tile([128, E], dt.float32)
            nc.vector.tensor_copy(lg[:], psl[:])
            vmax = rs.tile([128, 8], dt.float32)
            vidx = rs.tile([128, 8], dt.uint32)
            nc.vector.max_with_indices(vmax[:], vidx[:], lg[:])
            # renormalized top-2 weights: wA = sigmoid(l1-l2), wB = sigmoid(l2-l1)
            dAB = rs.tile([128, 2], dt.float32)
            nc.vector.tensor_tensor(out=dAB[:, 0:1], in0=vmax[:, 0:1],
                                    in1=vmax[:, 1:2], op=OP.subtract)
            nc.vector.tensor_tensor(out=dAB[:, 1:2], in0=vmax[:, 1:2],
                                    in1=vmax[:, 0:1], op=OP.subtract)
            nc.scalar.activation(wcomb[:, tt, :], dAB[:], AF.Sigmoid)
            # expert ids -> eall_d (pair-major: [0:TC]=top1, [TC:2TC]=top2)
            nc.sync.dma_start(
                eall_d[ts(tt, 128)].rearrange("(p one) -> p one", one=1),
                vidx[:, 0:1])
            nc.sync.dma_start(
                eall_d[TC + tt * 128: TC + (tt + 1) * 128]
                .rearrange("(p one) -> p one", one=1),
                vidx[:, 1:2])

    # =================== phase 2: rank scan + slot ids ===================
    with tc.tile_pool(name="scan", bufs=1) as sp, \
         tc.tile_pool(name="spsum", bufs=2, space="PSUM") as sps:
        ebc = sp.tile([E, 2 * TC], dt.uint32)
        nc.sync.dma_start(ebc[:], bass.AP(tensor=eall_d.tensor, offset=0,
                                          ap=[[0, E], [1, 2 * TC]]))
        ebcf = sp.tile([E, 2 * TC], dt.float32)
        nc.vector.tensor_copy(ebcf[:], ebc[:])
        iotaE = sp.tile([E, 1], dt.int32)
        nc.gpsimd.iota(iotaE[:], pattern=[[0, 1]], base=0, channel_multiplier=1)
        iotaEf = sp.tile([E, 1], dt.float32)
        nc.vector.tensor_copy(iotaEf[:], iotaE[:])
        mask8 = sp.tile([E, 2 * TC], dt.float32)
        nc.vector.tensor_scalar(mask8[:], ebcf[:], iotaEf[:, 0:1], None,
                                op0=OP.is_equal)
        zer8 = sp.tile([E, 2 * TC], dt.float32)
        nc.vector.memset(zer8[:], 0.0)
        pos8 = sp.tile([E, 2 * TC], dt.float32)
        nc.vector.tensor_tensor_scan(pos8[:], mask8[:], zer8[:], 0.0,
                                     op0=OP.add, op1=OP.add)
        nc.vector.tensor_tensor(out=mask8[:], in0=mask8[:], in1=pos8[:], op=OP.mult)
        ones8 = sp.tile([E, 1], dt.float32)
        nc.vector.memset(ones8[:], 1.0)
        d_sb = sp.tile([1, 2 * TC], dt.float32)
        for c8 in range(2 * TC // 512):
            psr = sps.tile([1, 512], dt.float32, space="PSUM")
            nc.tensor.matmul(psr[:], lhsT=ones8[:, 0:1],
                             rhs=mask8[:, ts(c8, 512)], start=True, stop=True)
            nc.vector.tensor_scalar(d_sb[0:1, ts(c8, 512)], psr[0:1, :], 1.0, None,
                                    op0=OP.subtract)
        erow = sp.tile([1, 2 * TC], dt.float32)
        nc.vector.tensor_scalar(erow[0:1, :], ebcf[0:1, :], float(CAP), None,
                                op0=OP.mult)
        nc.vector.tensor_tensor(out=d_sb[0:1, :], in0=d_sb[0:1, :],
                                in1=erow[0:1, :], op=OP.add)
        d_i = sp.tile([1, 2 * TC], dt.int32)
        nc.vector.tensor_copy(d_i[:], d_sb[:])
        nc.sync.dma_start(dall_d[:].rearrange("(one n) -> one n", one=1), d_i[:])

        # reload pair-major slot ids (also serves as token-major d1/d2)
        nc.sync.dma_start(dp_sb[:], dall_d[:].rearrange("(a p) -> p a", p=128))

        # zero src_d, then scatter token ids into slots
        zsc = sp.tile([128, E * STE], dt.int32)
        nc.vector.memset(zsc[:], 0)
        nc.sync.dma_start(src_d[:].rearrange("(a p) one -> p (a one)", p=128), zsc[:])
        tokv = sp.tile([128, 2, NT], dt.int32)
        nc.gpsimd.iota(tokv[:], pattern=[[0, 2], [128, NT]], base=0,
                       channel_multiplier=1)
        for pt in range(NPAIR):
            nc.gpsimd.indirect_dma_start(
                out=src_d[:],
                out_offset=bass.IndirectOffsetOnAxis(ap=dp_sb[:, pt:pt + 1], axis=0),
                in_=tokv[:, pt // NT, pt % NT: pt % NT + 1],
                in_offset=None,
                bounds_check=SLOTS - 1, oob_is_err=False)

        # slot-major source-token table
        nc.sync.dma_start(src_sb[:],
                          src_d[:].rearrange("(a p) one -> p (a one)", p=128))

    # =================== phase 3: per-expert sparse FFN ===================
    xgt_pool = _pool(name="xgt", bufs=BIG_BUFS)
    ht_pool = _pool(name="ht", bufs=BIG_BUFS)
    w13_pool = _pool(name="w13", bufs=4)
    w2_pool = _pool(name="w2", bufs=2)
    xg_pool = _pool(name="xg", bufs=3)
    sil_pool = _pool(name="sil", bufs=3)
    ygs_pool = _pool(name="ygs", bufs=3)
    psA_pool = _pool(name="psA", bufs=2, space="PSUM")
    psBig_pool = _pool(name="psBig", bufs=2, space="PSUM")

    for e in range(E):
        # ---- dispatch: gather + transpose into [h, slot] ----
        xgt = xgt_pool.tile([128, KH, CAP], MM_DT)
        for s in range(STE):
            xg = xg_pool.tile([128, H], MM_DT)
            nc.gpsimd.indirect_dma_start(
                out=xg[:], out_offset=None,
                in_=xb_d[:],
                in_offset=bass.IndirectOffsetOnAxis(
                    ap=src_sb[:, e * STE + s: e * STE + s + 1], axis=0))
            for jj in range(0, KH, 4):
                pst = psBig_pool.tile([128, 512], MM_DT, space="PSUM", tag="psbig")
                for j4 in range(4):
                    nc.tensor.transpose(pst[:, ts(j4, 128)],
                                        _mm_cast(xg[:, ts(jj + j4, 128)]),
                                        _mm_cast(ident[:]))
                nc.vector.tensor_copy(xgt[:, jj:jj + 4, ts(s, 128)], pst[:])

        # ---- stage A: hT = silu(w1 @ xgT) * (w3 @ xgT) ----
        ht = ht_pool.tile([128, KF, CAP], MM_DT)
        for fh in range(NFH):
            w1s = w13_pool.tile([128, KH, FSL], MM_DT, tag="w13")
            nc.sync.dma_start(w1s[:], w1_d[e, fh])
            w3s = w13_pool.tile([128, KH, FSL], MM_DT, tag="w13")
            nc.sync.dma_start(w3s[:], w3_d[e, fh])
            for fi in range(FSL // 128):
                f = fh * (FSL // 128) + fi
                for c in range(NCH):
                    ps1 = psA_pool.tile([128, CHUNK], dt.float32, space="PSUM")
                    for k in range(KH):
                        nc.tensor.matmul(ps1[:], lhsT=_mm_cast(w1s[:, k, ts(fi, 128)]),
                                         rhs=_mm_cast(xgt[:, k, ts(c, CHUNK)]),
                                         start=(k == 0), stop=(k == KH - 1))
                    ps3 = psA_pool.tile([128, CHUNK], dt.float32, space="PSUM")
                    for k in range(KH):
                        nc.tensor.matmul(ps3[:], lhsT=_mm_cast(w3s[:, k, ts(fi, 128)]),
                                         rhs=_mm_cast(xgt[:, k, ts(c, CHUNK)]),
                                         start=(k == 0), stop=(k == KH - 1))
                    sil = sil_pool.tile([128, CHUNK], MM_DT)
                    if USE_SILU:
                        nc.scalar.activation(sil[:], ps1[:], AF.Silu)
                    else:
                        # silu(x) = x * sigmoid(x); CoreSim lacks the Silu LUT
                        nc.scalar.activation(sil[:], ps1[:], AF.Sigmoid)
                        nc.vector.tensor_tensor(out=sil[:], in0=sil[:],
                                                in1=ps1[:], op=OP.mult)
                    nc.vector.tensor_tensor(out=ht[:, f, ts(c, CHUNK)],
                                            in0=sil[:], in1=ps3[:], op=OP.mult)

        # ---- stage B: yg = hT.T @ w2.T -> [slot, H] rows ----
        for hc in range(NW2):
            w2s = w2_pool.tile([128, KF, HSL], MM_DT)
            nc.sync.dma_start(w2s[:], w2_d[e, hc])
            for s in range(STE):
                psy = psBig_pool.tile([128, HSL], dt.float32, space="PSUM", tag="psbig")
                for k in range(KF):
                    nc.tensor.matmul(psy[:], lhsT=_mm_cast(ht[:, k, ts(s, 128)]),
                                     rhs=_mm_cast(w2s[:, k, :]),
                                     start=(k == 0), stop=(k == KF - 1))
                ygs = ygs_pool.tile([128, HSL], dt.float32)
                nc.vector.tensor_copy(ygs[:], psy[:])
                nc.sync.dma_start(
                    yg_d[e * CAP + s * 128: e * CAP + (s + 1) * 128,
                         hc * HSL:(hc + 1) * HSL],
                    ygs[:])

    # =================== phase 4: combine ===================
    with tc.tile_pool(name="fin", bufs=4) as fin, \
         tc.tile_pool(name="fout", bufs=3) as fout:
        for tt in range(NT):
            yA = fin.tile([128, H], dt.float32, tag="yab")
            nc.gpsimd.indirect_dma_start(
                out=yA[:], out_offset=None, in_=yg_d[:],
                in_offset=bass.IndirectOffsetOnAxis(ap=dp_sb[:, tt:tt + 1], axis=0))
            yB = fin.tile([128, H], dt.float32, tag="yab")
            nc.gpsimd.indirect_dma_start(
                out=yB[:], out_offset=None, in_=yg_d[:],
                in_offset=bass.IndirectOffsetOnAxis(
                    ap=dp_sb[:, NT + tt: NT + tt + 1], axis=0))
            ot = fout.tile([128, H], dt.float32)
            nc.vector.tensor_scalar(ot[:], yA[:], wcomb[:, tt, 0:1], None,
                                    op0=OP.mult)
            nc.vector.scalar_tensor_tensor(out=ot[:], in0=yB[:],
                                           scalar=wcomb[:, tt, 1:2], in1=ot[:],
                                           op0=OP.mult, op1=OP.add)
            nc.sync.dma_start(out_d[ts(tt, 128), :], ot[:])

    for p in reversed(_pools):
        p.release()


_NC_CACHE = None


def _get_nc():
    global _NC_CACHE
    if _NC_CACHE is None:
        _NC_CACHE = build_nc()
    return _NC_CACHE


def prepare_in_maps(hidden_states, gate_w, w1, w2, w3):
    x = np.ascontiguousarray(np.asarray(hidden_states, dtype=np.float32)
                             .reshape(T, H))
    gate_w = np.asarray(gate_w, dtype=np.float32)
    w1 = np.asarray(w1, dtype=np.float32)
    w2 = np.asarray(w2, dtype=np.float32)
    w3 = np.asarray(w3, dtype=np.float32)

    # weight swizzles (shared across cores)
    # w1s[e, fh, p, k, f] = w1[e, fh*FSL + f, k*128 + p]
    w1s = np.ascontiguousarray(
        w1.reshape(E, NFH, FSL, KH, 128).transpose(0, 1, 4, 3, 2)).astype(MM_NP)
    w3s = np.ascontiguousarray(
        w3.reshape(E, NFH, FSL, KH, 128).transpose(0, 1, 4, 3, 2)).astype(MM_NP)
    # w2s[e, hc, p, k, h] = w2[e, hc*HSL + h, k*128 + p]
    w2s = np.ascontiguousarray(
        w2.reshape(E, NW2, HSL, KF, 128).transpose(0, 1, 4, 3, 2)).astype(MM_NP)
    # gwt[p, k, e] = gate_w[e, k*128 + p]
    gwt = np.ascontiguousarray(
        gate_w.reshape(E, KH, 128).transpose(2, 1, 0))

    in_maps = []
    for c in range(NCORES):
        xs = x[c * TC:(c + 1) * TC]
        xt = np.ascontiguousarray(
            xs.reshape(TC, KH, 128).transpose(2, 1, 0))  # [p, k, t]
        in_maps.append({
            "xt": xt,
            "xb": np.ascontiguousarray(xs).astype(MM_NP),
            "gwt": gwt,
            "w1s": w1s,
            "w3s": w3s,
            "w2s": w2s,
        })
    return in_maps


def kernel(hidden_states, gate_w, w1, w2, w3):
    nc = _get_nc()
    in_maps = prepare_in_maps(hidden_states, gate_w, w1, w2, w3)
    res = run_bass_kernel_spmd(nc, in_maps, core_ids=list(range(NCORES)))
    out = np.concatenate([res.results[c]["out"] for c in range(NCORES)], axis=0)
    return out.reshape(B, S, H).astype(np.float32)


# revision 35
# speedup vs baseline: 1.0531x; 1.0531x over previous
"""Block-sparse MoE (top-2 of 8 experts, SwiGLU) for Trainium2, 8 NeuronCores.

Strategy: data-parallel over tokens (2048 tokens/core, no collectives),
with on-device routing and capacity-based sparse dispatch per core:

  1. Router: logits = x @ gate_w.T in fp32 on PE; top-2 via DVE max8;
     renormalized top-2 softmax weights computed as sigmoid(l_i - l_j).
  2. Dispatch: per-expert running ranks via a masked prefix-scan over the
     [8, 4096] one-hot pair matrix; slot id d = expert*CAP + rank; the
     inverse permutation (slot -> token) is built with an indirect-DMA
     scatter of token ids; tokens are gathered by row with indirect DMA
     and transposed on PE into [H, slot] layout for the FFN matmuls.
  3. FFN per expert (CAP=640 slots): hT = silu(w1 @ xgT) * (w3 @ xgT),
     yg = (hT.T @ w2.T) -> [slot, H] rows written to DRAM scratch.
  4. Combine: gather yg rows at each token's two slots, out = wA*yA + wB*yB.

Matmul compute dtype is a knob (bf16 / fp32 / f32r); router is always fp32.
"""
import os
import sys

if "/opt/trn_rl_repo" not in sys.path:
    sys.path.insert(0, "/opt/trn_rl_repo")

import numpy as np
import ml_dtypes

import concourse.bacc as bacc
import concourse.bass as bass
import concourse.mybir as mybir
import concourse.tile as tile
from concourse.bass import ts
from concourse.bass_utils import run_bass_kernel_spmd
from concourse.masks import make_identity

dt = mybir.dt

# ---- problem constants (hardcoded per spec) ----
B, S, H, F, E = 4, 4096, 1024, 2048, 8
T = B * S                  # 16384 tokens
NCORES = 8
TC = T // NCORES           # 2048 tokens per core
NT = TC // 128             # 16 token tiles
NPAIR = 2 * TC // 128      # 32 pair tiles
CAP = 640                  # per-(core,expert) slot capacity (max count is 565)
STE = CAP // 128           # 5 slot tiles per expert
SLOTS = E * CAP            # 5120
NCH = 2                    # slot chunks for stage-A psum (N<=512)
CHUNK = CAP // NCH         # 320
KH = H // 128              # 8 k-tiles over H
KF = F // 128              # 16 k-tiles over F
FT = F // 128              # 16 f tiles

MM_MODE = os.environ.get("MOE_MM_MODE", "bf16")  # bf16 | f32 | f32r
USE_SILU = os.environ.get("MOE_USE_SILU", "1") == "1"  # 0: sigmoid*x (sim-safe)
ACC_DT = dt.float32

if MM_MODE == "bf16":
    MM_DT = dt.bfloat16
    MM_NP = ml_dtypes.bfloat16
    NFH = 4                # F-slices for stage-A weight streaming
    NW2 = 2                # H-slices for stage-B weight streaming
    BIG_BUFS = 2           # xgt/ht double buffering
else:
    MM_DT = dt.float32
    MM_NP = np.float32
    NFH = 8
    NW2 = 4
    BIG_BUFS = 1
FSL = F // NFH             # stage-A weight slice width (f)
HSL = H // NW2             # stage-B weight slice width (h)


def _mm_cast(ap):
    """Bitcast fp32 APs to float32r for fast fp32 matmul when requested."""
    if MM_MODE == "f32r":
        return ap.bitcast(dt.float32r)
    return ap


def build_nc():
    nc = bacc.Bacc("TRN2", target_bir_lowering=False, debug=False)

    # ---- I/O ----
    xt_d = nc.dram_tensor("xt", [128, KH, TC], dt.float32, kind="ExternalInput").ap()
    xb_d = nc.dram_tensor("xb", [TC, H], MM_DT, kind="ExternalInput").ap()
    gwt_d = nc.dram_tensor("gwt", [128, KH, E], dt.float32, kind="ExternalInput").ap()
    w1_d = nc.dram_tensor("w1s", [E, NFH, 128, KH, FSL], MM_DT, kind="ExternalInput").ap()
    w3_d = nc.dram_tensor("w3s", [E, NFH, 128, KH, FSL], MM_DT, kind="ExternalInput").ap()
    w2_d = nc.dram_tensor("w2s", [E, NW2, 128, KF, HSL], MM_DT, kind="ExternalInput").ap()
    out_d = nc.dram_tensor("out", [TC, H], dt.float32, kind="ExternalOutput").ap()

    # ---- DRAM scratch ----
    eall_d = nc.dram_tensor("eall", [2 * TC], dt.uint32).ap()
    dall_d = nc.dram_tensor("dall", [2 * TC], dt.int32).ap()
    src_d = nc.dram_tensor("srcd", [SLOTS, 1], dt.int32).ap()
    yg_d = nc.dram_tensor("ygd", [SLOTS, H], dt.float32).ap()

    with tile.TileContext(nc) as tc:
        _emit(tc, nc, xt_d, xb_d, gwt_d, w1_d, w3_d, w2_d, out_d,
              eall_d, dall_d, src_d, yg_d)
    nc.compile()
    return nc


def _emit(tc, nc, xt_d, xb_d, gwt_d, w1_d, w3_d, w2_d, out_d,
          eall_d, dall_d, src_d, yg_d):
    AF = mybir.ActivationFunctionType
    OP = mybir.AluOpType

    _pools = []

    def _pool(**kw):
        p = tc.alloc_tile_pool(**kw)
        _pools.append(p)
        return p

    res = _pool(name="resident", bufs=1)
    # resident small tiles
    dcomb = res.tile([128, NT, 2], dt.float32)      # logit diffs (sigmoid deferred)
    ecomb = res.tile([128, 2, NT], dt.uint32)       # per-token top-2 expert ids
    dp_sb = res.tile([128, NPAIR], dt.int32)        # pair-major slot ids
    src_sb = res.tile([128, E * STE], dt.int32)     # slot-major source tokens
    ident = res.tile([128, 128], MM_DT)
    make_identity(nc, ident[:])

    # =================== phase 1: router ===================
    with tc.tile_pool(name="router", bufs=1) as rp, \
         tc.tile_pool(name="rsmall", bufs=4) as rs, \
         tc.tile_pool(name="rpsum", bufs=2, space="PSUM") as rps:
        xt = rp.tile([128, KH, TC], dt.float32)
        for xc in range(4):
            nc.sync.dma_start(xt[:, :, ts(xc, TC // 4)], xt_d[:, :, ts(xc, TC // 4)])
        gwt = rp.tile([128, KH, E], dt.float32)
        nc.sync.dma_start(gwt[:], gwt_d[:])

        for tt in range(NT):
            psl = rps.tile([128, E], dt.float32, space="PSUM")
            for k in range(KH):
                nc.tensor.matmul(psl[:], lhsT=xt[:, k, ts(tt, 128)],
                                 rhs=gwt[:, k, :], start=(k == 0), stop=(k == KH - 1))
            lg = rs.tile([128, E], dt.float32)
            nc.vector.tensor_copy(lg[:], psl[:])
            vmax = rs.tile([128, 8], dt.float32)
            vidx = rs.tile([128, 8], dt.uint32)
            nc.vector.max_with_indices(vmax[:], vidx[:], lg[:])
            # logit diffs; sigmoid deferred to the combine phase to keep the
            # ACT queue free for weight-stream DMAs during the prologue
            nc.vector.tensor_tensor(out=dcomb[:, tt, 0:1], in0=vmax[:, 0:1],
                                    in1=vmax[:, 1:2], op=OP.subtract)
            nc.vector.tensor_tensor(out=dcomb[:, tt, 1:2], in0=vmax[:, 1:2],
                                    in1=vmax[:, 0:1], op=OP.subtract)
            # expert ids -> resident buffer, flushed in one DMA below
            nc.gpsimd.tensor_copy(ecomb[:, :, tt], vidx[:, 0:2])
        # eall_d pair-major: [0:TC]=top1, [TC:2TC]=top2; flat = k*TC + tt*128 + p
        nc.sync.dma_start(
            eall_d[:].rearrange("(k a p) -> p k a", p=128, a=NT), ecomb[:])

    # =================== phase 2: rank scan + slot ids ===================
    with tc.tile_pool(name="scan", bufs=1) as sp, \
         tc.tile_pool(name="spsum", bufs=2, space="PSUM") as sps:
        ebc = sp.tile([E, 2 * TC], dt.uint32)
        nc.sync.dma_start(ebc[:], bass.AP(tensor=eall_d.tensor, offset=0,
                                          ap=[[0, E], [1, 2 * TC]]))
        ebcf = sp.tile([E, 2 * TC], dt.float32)
        nc.vector.tensor_copy(ebcf[:], ebc[:])
        iotaE = sp.tile([E, 1], dt.int32)
        nc.gpsimd.iota(iotaE[:], pattern=[[0, 1]], base=0, channel_multiplier=1)
        iotaEf = sp.tile([E, 1], dt.float32)
        nc.vector.tensor_copy(iotaEf[:], iotaE[:])
        mask8 = sp.tile([E, 2 * TC], dt.float32)
        nc.vector.tensor_scalar(mask8[:], ebcf[:], iotaEf[:, 0:1], None,
                                op0=OP.is_equal)
        zer8 = sp.tile([E, 2 * TC], dt.float32)
        nc.vector.memset(zer8[:], 0.0)
        pos8 = sp.tile([E, 2 * TC], dt.float32)
        nc.vector.tensor_tensor_scan(pos8[:], mask8[:], zer8[:], 0.0,
                                     op0=OP.add, op1=OP.add)
        nc.vector.tensor_tensor(out=mask8[:], in0=mask8[:], in1=pos8[:], op=OP.mult)
        ones8 = sp.tile([E, 1], dt.float32)
        nc.vector.memset(ones8[:], 1.0)
        d_sb = sp.tile([1, 2 * TC], dt.float32)
        for c8 in range(2 * TC // 512):
            psr = sps.tile([1, 512], dt.float32, space="PSUM")
            nc.tensor.matmul(psr[:], lhsT=ones8[:, 0:1],
                             rhs=mask8[:, ts(c8, 512)], start=True, stop=True)
            nc.vector.tensor_scalar(d_sb[0:1, ts(c8, 512)], psr[0:1, :], 1.0, None,
                                    op0=OP.subtract)
        # d = (e * CAP) + (pos - 1), fused
        nc.vector.scalar_tensor_tensor(out=d_sb[0:1, :], in0=ebcf[0:1, :],
                                       scalar=float(CAP), in1=d_sb[0:1, :],
                                       op0=OP.mult, op1=OP.add)
        d_i = sp.tile([1, 2 * TC], dt.int32)
        nc.vector.tensor_copy(d_i[:], d_sb[:])
        nc.sync.dma_start(dall_d[:].rearrange("(one n) -> one n", one=1), d_i[:])

        # reload pair-major slot ids (also serves as token-major d1/d2)
        nc.sync.dma_start(dp_sb[:], dall_d[:].rearrange("(a p) -> p a", p=128))

        # zero src_d, then scatter token ids into slots
        zsc = sp.tile([128, E * STE], dt.int32)
        nc.vector.memset(zsc[:], 0)
        nc.sync.dma_start(src_d[:].rearrange("(a p) one -> p (a one)", p=128), zsc[:])
        tokv = sp.tile([128, 2, NT], dt.int32)
        nc.gpsimd.iota(tokv[:], pattern=[[0, 2], [128, NT]], base=0,
                       channel_multiplier=1)
        # Touch inputs so their producer DMAs are sem-waited before the
        # critical section's entry barrier (deps are not tracked inside).
        probe = sp.tile([128, 1], dt.int32)
        nc.gpsimd.tensor_copy(probe[:], dp_sb[:, 0:1])
        nc.gpsimd.tensor_copy(probe[:], tokv[:, 0, 0:1])
        # The 32 pair-tile scatters write disjoint slots of src_d; under
        # normal tracking Tile chains them on DMA-completion sems (WAW on
        # src_d), costing ~4us each. Run them back-to-back in a critical
        # section with one manual completion semaphore.
        scat_sem = nc.alloc_semaphore("scat_sem")
        with tc.tile_critical():
            for pt in range(NPAIR):
                nc.gpsimd.indirect_dma_start(
                    out=src_d[:],
                    out_offset=bass.IndirectOffsetOnAxis(
                        ap=dp_sb[:, pt:pt + 1], axis=0),
                    in_=tokv[:, pt // NT, pt % NT: pt % NT + 1],
                    in_offset=None,
                    bounds_check=SLOTS - 1, oob_is_err=False).then_inc(scat_sem, 16)
            nc.sync.wait_ge(scat_sem, NPAIR * 16)

        # slot-major source-token table
        nc.sync.dma_start(src_sb[:],
                          src_d[:].rearrange("(a p) one -> p (a one)", p=128))

    # =================== phase 3: per-expert sparse FFN ===================
    xgt_pool = _pool(name="xgt", bufs=BIG_BUFS)
    ht_pool = _pool(name="ht", bufs=BIG_BUFS)
    w13_pool = _pool(name="w13", bufs=4)
    w2_pool = _pool(name="w2", bufs=2)
    xg_pool = _pool(name="xg", bufs=3)
    sil_pool = _pool(name="sil", bufs=3)
    ygs_pool = _pool(name="ygs", bufs=3)
    psA_pool = _pool(name="psA", bufs=2, space="PSUM")
    psBig_pool = _pool(name="psBig", bufs=2, space="PSUM")
    pst_pool = _pool(name="pst", bufs=2, space="PSUM")

    for e in range(E):
        # ---- dispatch: row gather + PE transpose into [h, slot] ----
        xgt = xgt_pool.tile([128, KH, CAP], MM_DT)
        for s in range(STE):
            xg = xg_pool.tile([128, H], MM_DT)
            nc.gpsimd.indirect_dma_start(
                out=xg[:], out_offset=None,
                in_=xb_d[:],
                in_offset=bass.IndirectOffsetOnAxis(
                    ap=src_sb[:, e * STE + s: e * STE + s + 1], axis=0))
            for jj in range(0, KH, 4):
                pst = pst_pool.tile([128, 512], MM_DT, space="PSUM")
                for j4 in range(4):
                    nc.tensor.transpose(pst[:, ts(j4, 128)],
                                        _mm_cast(xg[:, ts(jj + j4, 128)]),
                                        _mm_cast(ident[:]))
                nc.vector.tensor_copy(xgt[:, jj:jj + 4, ts(s, 128)], pst[:])

        # ---- stage A: hT = silu(w1 @ xgT) * (w3 @ xgT) ----
        ht = ht_pool.tile([128, KF, CAP], MM_DT)
        for fh in range(NFH):
            w1s = w13_pool.tile([128, KH, FSL], MM_DT, tag="w13")
            nc.scalar.dma_start(w1s[:], w1_d[e, fh])
            w3s = w13_pool.tile([128, KH, FSL], MM_DT, tag="w13")
            nc.scalar.dma_start(w3s[:], w3_d[e, fh])
            for fi in range(FSL // 128):
                f = fh * (FSL // 128) + fi
                for c in range(NCH):
                    ps1 = psA_pool.tile([128, CHUNK], dt.float32, space="PSUM")
                    for k in range(KH):
                        nc.tensor.matmul(ps1[:], lhsT=_mm_cast(w1s[:, k, ts(fi, 128)]),
                                         rhs=_mm_cast(xgt[:, k, ts(c, CHUNK)]),
                                         start=(k == 0), stop=(k == KH - 1))
                    ps3 = psA_pool.tile([128, CHUNK], dt.float32, space="PSUM")
                    for k in range(KH):
                        nc.tensor.matmul(ps3[:], lhsT=_mm_cast(w3s[:, k, ts(fi, 128)]),
                                         rhs=_mm_cast(xgt[:, k, ts(c, CHUNK)]),
                                         start=(k == 0), stop=(k == KH - 1))
                    sil = sil_pool.tile([128, CHUNK], MM_DT)
                    if USE_SILU:
                        nc.scalar.activation(sil[:], ps1[:], AF.Silu)
                    else:
                        # silu(x) = x * sigmoid(x); CoreSim lacks the Silu LUT
                        nc.scalar.activation(sil[:], ps1[:], AF.Sigmoid)
                        nc.vector.tensor_tensor(out=sil[:], in0=sil[:],
                                                in1=ps1[:], op=OP.mult)
                    nc.vector.tensor_tensor(out=ht[:, f, ts(c, CHUNK)],
                                            in0=sil[:], in1=ps3[:], op=OP.mult)

        # ---- stage B: yg = hT.T @ w2.T -> [slot, H] rows ----
        for hc in range(NW2):
            w2s = w2_pool.tile([128, KF, HSL], MM_DT)
            nc.scalar.dma_start(w2s[:], w2_d[e, hc])
            for s in range(STE):
                psy = psBig_pool.tile([128, HSL], dt.float32, space="PSUM", tag="psbig")
                for k in range(KF):
                    nc.tensor.matmul(psy[:], lhsT=_mm_cast(ht[:, k, ts(s, 128)]),
                                     rhs=_mm_cast(w2s[:, k, :]),
                                     start=(k == 0), stop=(k == KF - 1))
                ygs = ygs_pool.tile([128, HSL], dt.float32)
                nc.vector.tensor_copy(ygs[:], psy[:])
                nc.sync.dma_start(
                    yg_d[e * CAP + s * 128: e * CAP + (s + 1) * 128,
                         hc * HSL:(hc + 1) * HSL],
                    ygs[:])

    # =================== phase 4: combine ===================
    with tc.tile_pool(name="fin", bufs=4) as fin, \
         tc.tile_pool(name="fout", bufs=3) as fout:
        for tt in range(NT):
            wab = fin.tile([128, 2], dt.float32, tag="wab")
            nc.scalar.activation(wab[:], dcomb[:, tt, :], AF.Sigmoid)
            yA = fin.tile([128, H], dt.float32, tag="yab")
            nc.gpsimd.indirect_dma_start(
                out=yA[:], out_offset=None, in_=yg_d[:],
                in_offset=bass.IndirectOffsetOnAxis(ap=dp_sb[:, tt:tt + 1], axis=0))
            yB = fin.tile([128, H], dt.float32, tag="yab")
            nc.gpsimd.indirect_dma_start(
                out=yB[:], out_offset=None, in_=yg_d[:],
                in_offset=bass.IndirectOffsetOnAxis(
                    ap=dp_sb[:, NT + tt: NT + tt + 1], axis=0))
            ot = fout.tile([128, H], dt.float32)
            nc.vector.tensor_scalar(ot[:], yA[:], wab[:, 0:1], None,
                                    op0=OP.mult)
            nc.vector.scalar_tensor_tensor(out=ot[:], in0=yB[:],
                                           scalar=wab[:, 1:2], in1=ot[:],
                                           op0=OP.mult, op1=OP.add)
            nc.sync.dma_start(out_d[ts(tt, 128), :], ot[:])

    for p in reversed(_pools):
        p.release()


_NC_CACHE = None


def _get_nc():
    global _NC_CACHE
    if _NC_CACHE is None:
        _NC_CACHE = build_nc()
    return _NC_CACHE


def prepare_in_maps(hidden_states, gate_w, w1, w2, w3):
    x = np.ascontiguousarray(np.asarray(hidden_states, dtype=np.float32)
                             .reshape(T, H))
    gate_w = np.asarray(gate_w, dtype=np.float32)
    w1 = np.asarray(w1, dtype=np.float32)
    w2 = np.asarray(w2, dtype=np.float32)
    w3 = np.asarray(w3, dtype=np.float32)

    # weight swizzles (shared across cores)
    # w1s[e, fh, p, k, f] = w1[e, fh*FSL + f, k*128 + p]
    w1s = np.ascontiguousarray(
        w1.reshape(E, NFH, FSL, KH, 128).transpose(0, 1, 4, 3, 2)).astype(MM_NP)
    w3s = np.ascontiguousarray(
        w3.reshape(E, NFH, FSL, KH, 128).transpose(0, 1, 4, 3, 2)).astype(MM_NP)
    # w2s[e, hc, p, k, h] = w2[e, hc*HSL + h, k*128 + p]
    w2s = np.ascontiguousarray(
        w2.reshape(E, NW2, HSL, KF, 128).transpose(0, 1, 4, 3, 2)).astype(MM_NP)
    # gwt[p, k, e] = gate_w[e, k*128 + p]
    gwt = np.ascontiguousarray(
        gate_w.reshape(E, KH, 128).transpose(2, 1, 0))

    in_maps = []
    for c in range(NCORES):
        xs = x[c * TC:(c + 1) * TC]
        xt = np.ascontiguousarray(
            xs.reshape(TC, KH, 128).transpose(2, 1, 0))  # [p, k, t]
        in_maps.append({
            "xt": xt,
            "xb": np.ascontiguousarray(xs).astype(MM_NP),
            "gwt": gwt,
            "w1s": w1s,
            "w3s": w3s,
            "w2s": w2s,
        })
    return in_maps


def kernel(hidden_states, gate_w, w1, w2, w3):
    nc = _get_nc()
    in_maps = prepare_in_maps(hidden_states, gate_w, w1, w2, w3)
    res = run_bass_kernel_spmd(nc, in_maps, core_ids=list(range(NCORES)))
    out = np.concatenate([res.results[c]["out"] for c in range(NCORES)], axis=0)
    return out.reshape(B, S, H).astype(np.float32)
